# revision 1
# baseline (speedup 1.0000x reference)
"""Affinity-propagate (SPN) Trainium2 Bass kernel.

Computation (per batch element, see reference):
    w = g / conv3x3_ones(|g|)          # gates, [8, H, W], computed once
    d_{k+1} = max_c conv3x3_ones(w_c * d_k)   # 8 iterations

Distribution: pure data parallel, batch element b -> NeuronCore b (8 cores).

Per-core mapping:
  - H=352 rows live on SBUF partitions as 3 overlapping tiles
    (rows 0..127, 126..253, 252..351).  The 3x3 conv's H-direction sum is a
    tri-band matrix matmul on the tensor engine (contraction over the
    partition/H axis); output rows at tile seams that lack a cross-tile
    neighbour are invalid and are instead produced by the adjacent tile, with
    4 one-row SBUF->SBUF DMA "seam" copies per iteration.
  - The W-direction sum is folded into the same matmuls: 3 PSUM-accumulating
    matmuls with the moving operand shifted by -1/0/+1 columns (W is padded
    by one zero column on each side).
  - Work split: DVE computes p = w*d (fp32 in, float32r out, one op per
    (tile, channel-pair) via a stride-0 broadcast of d) and the incremental
    channel running-max; PE does all conv sums (float32r, 1 cycle/col at
    N>=256); ScalarE evacuates PSUM->SBUF; DMA does seam rows.
  - d is stored fp32, updated in place (trace order makes WAR/RAW safe);
    only p is rounded to float32r (~1e-4 relative per conv), keeping the
    final relative error ~3e-4.

Measured (8x trn2 NeuronCores via axon):
  - relative error vs fp32 jax reference: 3.03e-4
  - device execution: ~30-35 us per propagation iteration in good device
    state (N_ITERS=32/64 scaling inside one NEFF; the device drifts to
    ~1.8x slower states at times), i.e. ~250-350 us for the full
    8-iteration kernel.  Engine-ablation runs show the kernel is bound by
    the DVE stream (gating multiply + channel max, fp32 elementwise):
    cutting PE matmul work 3x or removing 95% of ScalarE evacuation work
    changes total time by <10%.
"""
from contextlib import ExitStack

import numpy as np

import concourse.bacc as bacc
import concourse.mybir as mybir
import concourse.tile as tile
from concourse.bass_utils import run_bass_kernel_spmd

F32 = mybir.dt.float32
F32R = mybir.dt.float32r

B, C, H, W = 8, 8, 352, 1216
WB = W + 2  # zero-padded width
N_ITERS = 8
N_CORES = 8
SKIP_SEAMS = False
P_BUFS = 3
PROP_BUFS = 6
T1_BUFS = 2
RM_BUFS = 2
N_SHIFTS = 3  # timing-ablation knob; must be 3 for correct results
SKIP_EVAC = False  # timing-ablation knob; must be False for correct results
PAIR_MULT = True  # one DVE mult per 2 channels (broadcast d over pair axis)

ROW_BASE = [0, 126, 252]       # first global row of each H tile
ROWS = [128, 128, 100]         # partitions used by each H tile
CHUNKS = [(0, 512), (512, 448), (960, 256)]  # (start col, width); >=256 for f32r speed
CHUNKS_SMALL_FIRST = False


def _build_nc():
    nc = bacc.Bacc("TRN2", target_bir_lowering=False, debug=False,
                   num_devices=N_CORES)
    g = nc.dram_tensor("g", [C, H, W], F32, kind="ExternalInput").ap()
    d_in = nc.dram_tensor("d", [H, W], F32, kind="ExternalInput").ap()
    band = nc.dram_tensor("band", [128, 128], F32R, kind="ExternalInput").ap()
    out = nc.dram_tensor("out", [H, W], F32, kind="ExternalOutput").ap()

    with tile.TileContext(nc) as tc, ExitStack() as ctx:
        pw = ctx.enter_context(tc.tile_pool(name="w", bufs=1))
        pd = ctx.enter_context(tc.tile_pool(name="d", bufs=1))
        pc = ctx.enter_context(tc.tile_pool(name="const", bufs=1))
        pp = ctx.enter_context(tc.tile_pool(name="p", bufs=P_BUFS))
        pprop = ctx.enter_context(tc.tile_pool(name="prop", bufs=PROP_BUFS))
        ptree1 = ctx.enter_context(tc.tile_pool(name="tree1", bufs=T1_BUFS))
        prm = ctx.enter_context(tc.tile_pool(name="rm", bufs=RM_BUFS))
        psum = ctx.enter_context(tc.tile_pool(name="psum", bufs=8, space="PSUM"))

        A = pc.tile([128, 128], F32R, tag="band", name="bandt")
        nc.sync.dma_start(A[:], band[:])

        wt = [pw.tile([128, C, WB], F32, tag=f"w{t}", name=f"w{t}")
              for t in range(3)]
        dt_ = [pd.tile([128, WB], F32, tag=f"d{t}", name=f"d{t}")
               for t in range(3)]

        # ---- zero pad columns, load inputs ----
        for t in range(3):
            R, rb = ROWS[t], ROW_BASE[t]
            nc.vector.memset(wt[t][:, :, 0:1], 0.0)
            nc.vector.memset(wt[t][:, :, WB - 1:WB], 0.0)
            nc.vector.memset(dt_[t][:, 0:1], 0.0)
            nc.vector.memset(dt_[t][:, WB - 1:WB], 0.0)
            nc.sync.dma_start(
                wt[t][0:R, :, 1:W + 1],
                g[:, rb:rb + R, :].rearrange("c r w -> r c w"))
            nc.sync.dma_start(dt_[t][0:R, 1:W + 1], d_in[rb:rb + R, :])

        # ---- phase 0: w = g / conv3x3_ones(|g|) ----
        for t in range(3):
            R = ROWS[t]
            for c in range(C):
                p = pp.tile([128, WB], F32R, tag="p", name="p")
                nc.scalar.activation(p[0:R, :], wt[t][0:R, c, :],
                                     mybir.ActivationFunctionType.Abs)
                s_buf = ptree1.tile([128, W], F32, tag="t1", name="sbuf_")
                for (J, N) in CHUNKS:
                    ps = psum.tile([128, 512], F32, tag="ps", name="ps")
                    for s in range(3):
                        nc.tensor.matmul(ps[0:R, 0:N], A[0:R, 0:R],
                                         p[0:R, J + s:J + s + N],
                                         start=(s == 0), stop=(s == 2))
                    nc.scalar.copy(s_buf[0:R, J:J + N], ps[0:R, 0:N])
                rcp = ptree1.tile([128, W], F32, tag="t1", name="rcp")
                nc.vector.reciprocal_approx_fast(out=rcp[0:R, :],
                                                 in_=s_buf[0:R, :])
                nc.vector.tensor_mul(wt[t][0:R, c, 1:W + 1],
                                     wt[t][0:R, c, 1:W + 1], rcp[0:R, :])
        # w seam rows
        nc.sync.dma_start(wt[0][127:128, :, 1:W + 1], wt[1][1:2, :, 1:W + 1])
        nc.sync.dma_start(wt[1][0:1, :, 1:W + 1], wt[0][126:127, :, 1:W + 1])
        nc.sync.dma_start(wt[1][127:128, :, 1:W + 1], wt[2][1:2, :, 1:W + 1])
        nc.sync.dma_start(wt[2][0:1, :, 1:W + 1], wt[1][126:127, :, 1:W + 1])

        # ---- phase 1: 8 propagation iterations ----
        for k in range(N_ITERS):
            for t in range(3):
                R = ROWS[t]
                props = []
                pairs = {}
                for c in range(C):
                    if PAIR_MULT:
                        if c % 2 == 0:
                            p2 = pp.tile([128, 2, WB], F32R, tag="p", name="p2")
                            dbc = dt_[t][0:R, :].unsqueeze(1).broadcast_to(
                                [R, 2, WB])
                            nc.vector.tensor_mul(p2[0:R, :, :],
                                                 wt[t][0:R, c:c + 2, :], dbc)
                            pairs[c] = p2
                        p = pairs[c - (c % 2)][:, c % 2]
                    else:
                        p = pp.tile([128, WB], F32R, tag="p", name="p")
                        nc.vector.tensor_mul(p[0:R, :], wt[t][0:R, c, :],
                                             dt_[t][0:R, :])
                    prop = pprop.tile([128, W], F32, tag="prop", name="prop")
                    ch = CHUNKS[::-1] if CHUNKS_SMALL_FIRST else CHUNKS
                    for (J, N) in ch:
                        ps = psum.tile([128, 512], F32, tag="ps", name="ps")
                        for s in range(N_SHIFTS):
                            nc.tensor.matmul(ps[0:R, 0:N], A[0:R, 0:R],
                                             p[0:R, J + s:J + s + N],
                                             start=(s == 0),
                                             stop=(s == N_SHIFTS - 1))
                        if SKIP_EVAC:
                            nc.scalar.copy(prop[0:R, J:J + 8], ps[0:R, 0:8])
                        else:
                            nc.scalar.copy(prop[0:R, J:J + N], ps[0:R, 0:N])
                    props.append(prop)
                    # incremental channel max; last step writes d in place
                    # (junk seam rows fixed by the seam DMAs below)
                    if c == 1:
                        rm = prm.tile([128, W], F32, tag="rm", name="rm")
                        nc.vector.tensor_max(rm[0:R, :], props[0][0:R, :],
                                             props[1][0:R, :])
                    elif c in (2, 3, 4, 5, 6):
                        nc.vector.tensor_max(rm[0:R, :], rm[0:R, :],
                                             props[c][0:R, :])
                    elif c == 7:
                        nc.vector.tensor_max(dt_[t][0:R, 1:W + 1],
                                             rm[0:R, :], props[7][0:R, :])
            # seam rows
            if not SKIP_SEAMS:
                nc.sync.dma_start(dt_[0][127:128, 1:W + 1], dt_[1][1:2, 1:W + 1])
                nc.sync.dma_start(dt_[1][0:1, 1:W + 1], dt_[0][126:127, 1:W + 1])
                nc.sync.dma_start(dt_[1][127:128, 1:W + 1], dt_[2][1:2, 1:W + 1])
                nc.sync.dma_start(dt_[2][0:1, 1:W + 1], dt_[1][126:127, 1:W + 1])

        nc.sync.dma_start(out[0:128, :], dt_[0][0:128, 1:W + 1])
        nc.sync.dma_start(out[128:254, :], dt_[1][2:128, 1:W + 1])
        nc.sync.dma_start(out[254:352, :], dt_[2][2:100, 1:W + 1])

    nc.compile()
    return nc


def _band_matrix():
    a = np.zeros((128, 128), dtype=np.float32)
    idx = np.arange(128)
    a[idx, idx] = 1.0
    a[idx[:-1], idx[:-1] + 1] = 1.0
    a[idx[1:], idx[1:] - 1] = 1.0
    return a


_NC_CACHE = None


def kernel(guidance: np.ndarray, blur_depth: np.ndarray) -> np.ndarray:
    """Full inputs in, full output out. Shards batch across 8 NeuronCores."""
    global _NC_CACHE
    guidance = np.asarray(guidance, dtype=np.float32)
    blur_depth = np.asarray(blur_depth, dtype=np.float32)
    assert guidance.shape == (B, C, H, W), guidance.shape
    assert blur_depth.shape == (B, 1, H, W), blur_depth.shape
    if _NC_CACHE is None:
        _NC_CACHE = _build_nc()
    nc = _NC_CACHE
    band = _band_matrix()
    in_maps = [
        {
            "g": np.ascontiguousarray(guidance[b], dtype=np.float32),
            "d": np.ascontiguousarray(blur_depth[b, 0], dtype=np.float32),
            "band": band,
        }
        for b in range(B)
    ]
    res = run_bass_kernel_spmd(nc, in_maps, core_ids=list(range(N_CORES)))
    out = np.stack([res.results[b]["out"] for b in range(B)])[:, None]
    return out.astype(np.float32)



# revision 3
# speedup vs baseline: 1.3276x; 1.3276x over previous
"""Affinity-propagate (SPN) Trainium2 Bass kernel, fp16 pipeline.

Computation (per batch element, see reference):
    w = g / conv3x3_ones(|g|)          # gates, [8, H, W], computed once
    d_{k+1} = max_c conv3x3_ones(w_c * d_k)   # 8 iterations

Distribution: pure data parallel, batch element b -> NeuronCore b (8 cores).

Per-core mapping (H=352 rows as 3 overlapping 128-row tiles):
  - All gate/depth data is fp16: DVE tensor_tensor ops run in 2x_1p mode
    (2 elem/cycle), halving the vector-engine time vs fp32.
  - p = w * d is ONE DVE mult per tile ([128, 8, 1218], d broadcast over the
    channel axis; innermost stride 1 keeps the 2x mode).
  - 3x3 conv = tri-band matmul over the H/partition axis (fp16 stationary)
    x 3 PSUM-accumulated W-shifts; W is chunked 4x304 so each channel's
    conv occupies 4 PSUM banks and is evacuated PSUM->SBUF fp16 by ONE
    ScalarE copy (multi-bank AP), amortizing the per-op overhead.
  - channel max: 7 DVE tensor_max ops (fp16 2x), last one writes d in place.
  - seam rows between H tiles are fixed with 1-row SBUF->SBUF DMAs.
"""
from contextlib import ExitStack

import numpy as np

import concourse.bacc as bacc
import concourse.mybir as mybir
import concourse.tile as tile
from concourse.bass_utils import run_bass_kernel_spmd

F32 = mybir.dt.float32
F16 = mybir.dt.float16

B, C, H, W = 8, 8, 352, 1216
WB = W + 2  # zero-padded width
N_ITERS = 8
N_CORES = 8

ROW_BASE = [0, 126, 252]       # first global row of each H tile
ROWS = [128, 128, 100]         # partitions used by each H tile
NCHUNK = 4
CW = 304                        # chunk width; 4 chunks of 304 = 1216


def _build_nc():
    nc = bacc.Bacc("TRN2", target_bir_lowering=False, debug=False,
                   num_devices=N_CORES)
    g = nc.dram_tensor("g", [C, H, W], F32, kind="ExternalInput").ap()
    d_in = nc.dram_tensor("d", [H, W], F32, kind="ExternalInput").ap()
    band = nc.dram_tensor("band", [128, 128], F16, kind="ExternalInput").ap()
    out = nc.dram_tensor("out", [H, W], F32, kind="ExternalOutput").ap()

    with tile.TileContext(nc) as tc, ExitStack() as ctx:
        pw = ctx.enter_context(tc.tile_pool(name="w", bufs=1))
        pd = ctx.enter_context(tc.tile_pool(name="d", bufs=1))
        pc = ctx.enter_context(tc.tile_pool(name="const", bufs=1))
        pg = ctx.enter_context(tc.tile_pool(name="g32", bufs=3))
        pa = ctx.enter_context(tc.tile_pool(name="abs16", bufs=2))
        pr = ctx.enter_context(tc.tile_pool(name="r32", bufs=2))
        pp = ctx.enter_context(tc.tile_pool(name="p", bufs=2))
        pprop = ctx.enter_context(tc.tile_pool(name="prop", bufs=10))
        prm = ctx.enter_context(tc.tile_pool(name="rm", bufs=2))
        psum = ctx.enter_context(tc.tile_pool(name="psum", bufs=2,
                                              space="PSUM"))

        A = pc.tile([128, 128], F16, tag="band", name="bandt")
        nc.sync.dma_start(A[:], band[:])

        wt = [pw.tile([128, C, WB], F16, tag=f"w{t}", name=f"w{t}")
              for t in range(3)]
        dt_ = [pd.tile([128, WB], F16, tag=f"d{t}", name=f"d{t}")
               for t in range(3)]

        # ---- load depth (fp32 staging -> fp16), zero pads ----
        for t in range(3):
            R, rb = ROWS[t], ROW_BASE[t]
            nc.vector.memset(wt[t][:, :, 0:1], 0.0)
            nc.vector.memset(wt[t][:, :, WB - 1:WB], 0.0)
            nc.vector.memset(dt_[t][:, 0:1], 0.0)
            nc.vector.memset(dt_[t][:, WB - 1:WB], 0.0)
            d32 = pg.tile([128, W], F32, tag="g32", name="d32")
            nc.sync.dma_start(d32[0:R, :], d_in[rb:rb + R, :])
            nc.vector.tensor_copy(dt_[t][0:R, 1:W + 1], d32[0:R, :])

        # ---- phase 0: w = g / conv3x3_ones(|g|) ----
        for t in range(3):
            R, rb = ROWS[t], ROW_BASE[t]
            for c in range(C):
                g32 = pg.tile([128, W], F32, tag="g32", name="g32")
                nc.sync.dma_start(g32[0:R, :], g[c, rb:rb + R, :])
                a16 = pa.tile([128, WB], F16, tag="a16", name="a16")
                nc.vector.memset(a16[0:R, 0:1], 0.0)
                nc.vector.memset(a16[0:R, WB - 1:WB], 0.0)
                nc.scalar.activation(a16[0:R, 1:W + 1], g32[0:R, :],
                                     mybir.ActivationFunctionType.Abs)
                ps = psum.tile([128, NCHUNK, 512], F32, tag="ps", name="ps")
                for k in range(NCHUNK):
                    for s in range(3):
                        nc.tensor.matmul(ps[0:R, k, 0:CW], A[0:R, 0:R],
                                         a16[0:R, k * CW + s:k * CW + s + CW],
                                         start=(s == 0), stop=(s == 2))
                r32 = pr.tile([128, NCHUNK, CW], F32, tag="r32", name="r32")
                nc.vector.reciprocal_approx_fast(
                    out=r32[0:R, :, :], in_=ps[0:R, :, 0:CW])
                nc.vector.tensor_mul(
                    wt[t][0:R, c, 1:W + 1], g32[0:R, :],
                    r32[0:R, :, :].rearrange("p a b -> p (a b)"))
        # w seam rows
        nc.sync.dma_start(wt[0][127:128, :, 1:W + 1], wt[1][1:2, :, 1:W + 1])
        nc.sync.dma_start(wt[1][0:1, :, 1:W + 1], wt[0][126:127, :, 1:W + 1])
        nc.sync.dma_start(wt[1][127:128, :, 1:W + 1], wt[2][1:2, :, 1:W + 1])
        nc.sync.dma_start(wt[2][0:1, :, 1:W + 1], wt[1][126:127, :, 1:W + 1])

        # ---- phase 1: 8 propagation iterations ----
        for k in range(N_ITERS):
            for t in range(3):
                R = ROWS[t]
                p16 = pp.tile([128, C, WB], F16, tag="p", name="p16")
                dbc = dt_[t][0:R, :].unsqueeze(1).broadcast_to([R, C, WB])
                nc.vector.tensor_mul(p16[0:R, :, :], wt[t][0:R, :, :], dbc)
                props = []
                for c in range(C):
                    ps = psum.tile([128, NCHUNK, 512], F32, tag="ps",
                                   name="ps")
                    for kk in range(NCHUNK):
                        for s in range(3):
                            nc.tensor.matmul(
                                ps[0:R, kk, 0:CW], A[0:R, 0:R],
                                p16[0:R, c, kk * CW + s:kk * CW + s + CW],
                                start=(s == 0), stop=(s == 2))
                    prop = pprop.tile([128, W], F16, tag="prop", name="prop")
                    nc.scalar.copy(
                        prop[0:R, :].rearrange("p (a b) -> p a b", a=NCHUNK),
                        ps[0:R, :, 0:CW])
                    props.append(prop)
                    # incremental channel max; last step writes d in place
                    if c == 1:
                        rm = prm.tile([128, W], F16, tag="rm", name="rm")
                        nc.vector.tensor_max(rm[0:R, :], props[0][0:R, :],
                                             props[1][0:R, :])
                    elif c in (2, 3, 4, 5, 6):
                        nc.vector.tensor_max(rm[0:R, :], rm[0:R, :],
                                             props[c][0:R, :])
                    elif c == 7:
                        nc.vector.tensor_max(dt_[t][0:R, 1:W + 1],
                                             rm[0:R, :], props[7][0:R, :])
            # seam rows
            nc.sync.dma_start(dt_[0][127:128, 1:W + 1], dt_[1][1:2, 1:W + 1])
            nc.sync.dma_start(dt_[1][0:1, 1:W + 1], dt_[0][126:127, 1:W + 1])
            nc.sync.dma_start(dt_[1][127:128, 1:W + 1], dt_[2][1:2, 1:W + 1])
            nc.sync.dma_start(dt_[2][0:1, 1:W + 1], dt_[1][126:127, 1:W + 1])

        # ---- output: fp16 -> fp32 staging -> HBM ----
        outspec = [(0, 0, 128), (1, 2, 128), (2, 2, 100)]
        for t, r0, r1 in outspec:
            o32 = pg.tile([128, W], F32, tag="g32", name="o32")
            nc.vector.tensor_copy(o32[0:ROWS[t], :], dt_[t][0:ROWS[t], 1:W + 1])
            gb = ROW_BASE[t] + r0
            nc.sync.dma_start(out[gb:gb + (r1 - r0), :], o32[r0:r1, :])

    nc.compile()
    return nc


def _band_matrix():
    a = np.zeros((128, 128), dtype=np.float16)
    idx = np.arange(128)
    a[idx, idx] = 1.0
    a[idx[:-1], idx[:-1] + 1] = 1.0
    a[idx[1:], idx[1:] - 1] = 1.0
    return a


_NC_CACHE = None


def kernel(guidance: np.ndarray, blur_depth: np.ndarray) -> np.ndarray:
    """Full inputs in, full output out. Shards batch across 8 NeuronCores."""
    global _NC_CACHE
    guidance = np.asarray(guidance, dtype=np.float32)
    blur_depth = np.asarray(blur_depth, dtype=np.float32)
    assert guidance.shape == (B, C, H, W), guidance.shape
    assert blur_depth.shape == (B, 1, H, W), blur_depth.shape
    if _NC_CACHE is None:
        _NC_CACHE = _build_nc()
    nc = _NC_CACHE
    band = _band_matrix()
    in_maps = [
        {
            "g": np.ascontiguousarray(guidance[b], dtype=np.float32),
            "d": np.ascontiguousarray(blur_depth[b, 0], dtype=np.float32),
            "band": band,
        }
        for b in range(B)
    ]
    res = run_bass_kernel_spmd(nc, in_maps, core_ids=list(range(N_CORES)))
    out = np.stack([res.results[b]["out"] for b in range(B)])[:, None]
    return out.astype(np.float32)


# revision 6
# speedup vs baseline: 1.3764x; 1.0368x over previous
"""Affinity-propagate (SPN) Trainium2 Bass kernel, fp16 pipeline.

Computation (per batch element, see reference):
    w = g / conv3x3_ones(|g|)          # gates, [8, H, W], computed once
    d_{k+1} = max_c conv3x3_ones(w_c * d_k)   # 8 iterations

Distribution: pure data parallel, batch element b -> NeuronCore b (8 cores).

Per-core mapping (H=352 rows as 3 overlapping 128-row tiles):
  - All gate/depth data is fp16: DVE tensor_tensor ops run in 2x_1p mode
    (2 elem/cycle), halving the vector-engine time vs fp32.
  - p = w * d is ONE DVE mult per tile ([128, 8, WB], d broadcast over the
    channel axis; innermost stride 1 keeps the 2x mode).
  - 3x3 conv = tri-band matmul over the H/partition axis (fp16 stationary)
    x 3 PSUM-accumulated W-shifts; W is chunked 3x406 so each channel's
    conv occupies 3 PSUM banks and is evacuated PSUM->SBUF fp16 by ONE
    ScalarE copy (multi-bank AP), amortizing the per-op overhead.
  - channel max: 7 DVE tensor_max ops (fp16 2x), last one writes d in place.
  - seam rows between H tiles are fixed with 1-row SBUF->SBUF DMAs.
  - input g loads are spread across the Sync/Scalar/Vector DMA queues and
    staged 6 deep so the load pipeline stays ahead of gate normalization.
"""
from contextlib import ExitStack

import numpy as np

import concourse.bacc as bacc
import concourse.mybir as mybir
import concourse.tile as tile
from concourse.bass_utils import run_bass_kernel_spmd

F32 = mybir.dt.float32
F16 = mybir.dt.float16

B, C, H, W = 8, 8, 352, 1216
NCHUNK = 3
CW = 406                        # chunk width; 3 chunks of 406 = 1218 >= W
WB = NCHUNK * CW + 2            # 1220: [0]=pad, 1..1216 data, 1217+ pad
N_ITERS = 8
N_CORES = 8

ROW_BASE = [0, 126, 252]       # first global row of each H tile
ROWS = [128, 128, 100]         # partitions used by each H tile


def _build_nc():
    nc = bacc.Bacc("TRN2", target_bir_lowering=False, debug=False,
                   num_devices=N_CORES)
    g = nc.dram_tensor("g", [C, H, W], F32, kind="ExternalInput").ap()
    d_in = nc.dram_tensor("d", [H, W], F32, kind="ExternalInput").ap()
    band = nc.dram_tensor("band", [128, 128], F16, kind="ExternalInput").ap()
    out = nc.dram_tensor("out", [H, W], F32, kind="ExternalOutput").ap()

    with tile.TileContext(nc) as tc, ExitStack() as ctx:
        pw = ctx.enter_context(tc.tile_pool(name="w", bufs=1))
        pd = ctx.enter_context(tc.tile_pool(name="d", bufs=1))
        pc = ctx.enter_context(tc.tile_pool(name="const", bufs=1))
        pg = ctx.enter_context(tc.tile_pool(name="g32", bufs=6))
        pa = ctx.enter_context(tc.tile_pool(name="abs16", bufs=2))
        pr = ctx.enter_context(tc.tile_pool(name="r32", bufs=2))
        pp = ctx.enter_context(tc.tile_pool(name="p", bufs=2))
        pprop = ctx.enter_context(tc.tile_pool(name="prop", bufs=9))
        prm = ctx.enter_context(tc.tile_pool(name="rm", bufs=2))
        psum = ctx.enter_context(tc.tile_pool(name="psum", bufs=2,
                                              space="PSUM"))

        A = pc.tile([128, 128], F16, tag="band", name="bandt")
        nc.sync.dma_start(A[:], band[:])

        wt = [pw.tile([128, C, WB], F16, tag=f"w{t}", name=f"w{t}")
              for t in range(3)]
        dt_ = [pd.tile([128, WB], F16, tag=f"d{t}", name=f"d{t}")
               for t in range(3)]

        # ---- load depth (fp32 staging -> fp16), zero pads ----
        for t in range(3):
            R, rb = ROWS[t], ROW_BASE[t]
            nc.vector.memset(wt[t][:, :, 0:1], 0.0)
            nc.vector.memset(wt[t][:, :, W + 1:WB], 0.0)
            nc.vector.memset(dt_[t][:, 0:1], 0.0)
            nc.vector.memset(dt_[t][:, W + 1:WB], 0.0)
            d32 = pg.tile([128, W], F32, tag="g32", name="d32")
            nc.sync.dma_start(d32[0:R, :], d_in[rb:rb + R, :])
            nc.vector.tensor_copy(dt_[t][0:R, 1:W + 1], d32[0:R, :])

        # ---- phase 0: w = g / conv3x3_ones(|g|) ----
        dma_engines = [nc.sync, nc.scalar]
        for t in range(3):
            R, rb = ROWS[t], ROW_BASE[t]
            for c in range(C):
                g32 = pg.tile([128, W], F32, tag="g32", name="g32")
                dma_engines[(t * C + c) % 2].dma_start(
                    g32[0:R, :], g[c, rb:rb + R, :])
                a16 = pa.tile([128, WB], F16, tag="a16", name="a16")
                nc.vector.memset(a16[0:R, 0:1], 0.0)
                nc.vector.memset(a16[0:R, W + 1:WB], 0.0)
                nc.scalar.activation(a16[0:R, 1:W + 1], g32[0:R, :],
                                     mybir.ActivationFunctionType.Abs)
                ps = psum.tile([128, NCHUNK, 512], F32, tag="ps", name="ps")
                for k in range(NCHUNK):
                    for s in range(3):
                        nc.tensor.matmul(ps[0:R, k, 0:CW], A[0:R, 0:R],
                                         a16[0:R, k * CW + s:k * CW + s + CW],
                                         start=(s == 0), stop=(s == 2))
                r32 = pr.tile([128, NCHUNK, CW], F32, tag="r32", name="r32")
                nc.vector.reciprocal_approx_fast(
                    out=r32[0:R, :, :], in_=ps[0:R, :, 0:CW])
                nc.vector.tensor_mul(
                    wt[t][0:R, c, 1:W + 1], g32[0:R, :],
                    r32[0:R, :, :].rearrange("p a b -> p (a b)")[:, 0:W])
        # w seam rows
        nc.sync.dma_start(wt[0][127:128, :, 1:W + 1], wt[1][1:2, :, 1:W + 1])
        nc.sync.dma_start(wt[1][0:1, :, 1:W + 1], wt[0][126:127, :, 1:W + 1])
        nc.sync.dma_start(wt[1][127:128, :, 1:W + 1], wt[2][1:2, :, 1:W + 1])
        nc.sync.dma_start(wt[2][0:1, :, 1:W + 1], wt[1][126:127, :, 1:W + 1])

        # ---- phase 1: 8 propagation iterations ----
        for k in range(N_ITERS):
            for t in range(3):
                R = ROWS[t]
                p16 = pp.tile([128, C, WB], F16, tag="p", name="p16")
                dbc = dt_[t][0:R, :].unsqueeze(1).broadcast_to([R, C, WB])
                nc.vector.tensor_mul(p16[0:R, :, :], wt[t][0:R, :, :], dbc)
                props = []
                for c in range(C):
                    ps = psum.tile([128, NCHUNK, 512], F32, tag="ps",
                                   name="ps")
                    for kk in range(NCHUNK):
                        for s in range(3):
                            nc.tensor.matmul(
                                ps[0:R, kk, 0:CW], A[0:R, 0:R],
                                p16[0:R, c, kk * CW + s:kk * CW + s + CW],
                                start=(s == 0), stop=(s == 2))
                    prop = pprop.tile([128, NCHUNK * CW], F16, tag="prop",
                                      name="prop")
                    nc.scalar.copy(
                        prop[0:R, :].rearrange("p (a b) -> p a b", a=NCHUNK),
                        ps[0:R, :, 0:CW])
                    props.append(prop)
                    # incremental channel max; last step writes d in place
                    if c == 1:
                        rm = prm.tile([128, W], F16, tag="rm", name="rm")
                        nc.vector.tensor_max(rm[0:R, :], props[0][0:R, 0:W],
                                             props[1][0:R, 0:W])
                    elif c in (2, 3, 4, 5, 6):
                        nc.vector.tensor_max(rm[0:R, :], rm[0:R, :],
                                             props[c][0:R, 0:W])
                    elif c == 7:
                        nc.vector.tensor_max(dt_[t][0:R, 1:W + 1],
                                             rm[0:R, :], props[7][0:R, 0:W])
            # seam rows
            nc.sync.dma_start(dt_[0][127:128, 1:W + 1], dt_[1][1:2, 1:W + 1])
            nc.sync.dma_start(dt_[1][0:1, 1:W + 1], dt_[0][126:127, 1:W + 1])
            nc.sync.dma_start(dt_[1][127:128, 1:W + 1], dt_[2][1:2, 1:W + 1])
            nc.sync.dma_start(dt_[2][0:1, 1:W + 1], dt_[1][126:127, 1:W + 1])

        # ---- output: fp16 -> fp32 staging -> HBM ----
        outspec = [(0, 0, 128), (1, 2, 128), (2, 2, 100)]
        for t, r0, r1 in outspec:
            o32 = pg.tile([128, W], F32, tag="g32", name="o32")
            nc.vector.tensor_copy(o32[0:ROWS[t], :], dt_[t][0:ROWS[t], 1:W + 1])
            gb = ROW_BASE[t] + r0
            nc.sync.dma_start(out[gb:gb + (r1 - r0), :], o32[r0:r1, :])

    nc.compile()
    return nc


def _band_matrix():
    a = np.zeros((128, 128), dtype=np.float16)
    idx = np.arange(128)
    a[idx, idx] = 1.0
    a[idx[:-1], idx[:-1] + 1] = 1.0
    a[idx[1:], idx[1:] - 1] = 1.0
    return a


_NC_CACHE = None


def kernel(guidance: np.ndarray, blur_depth: np.ndarray) -> np.ndarray:
    """Full inputs in, full output out. Shards batch across 8 NeuronCores."""
    global _NC_CACHE
    guidance = np.asarray(guidance, dtype=np.float32)
    blur_depth = np.asarray(blur_depth, dtype=np.float32)
    assert guidance.shape == (B, C, H, W), guidance.shape
    assert blur_depth.shape == (B, 1, H, W), blur_depth.shape
    if _NC_CACHE is None:
        _NC_CACHE = _build_nc()
    nc = _NC_CACHE
    band = _band_matrix()
    in_maps = [
        {
            "g": np.ascontiguousarray(guidance[b], dtype=np.float32),
            "d": np.ascontiguousarray(blur_depth[b, 0], dtype=np.float32),
            "band": band,
        }
        for b in range(B)
    ]
    res = run_bass_kernel_spmd(nc, in_maps, core_ids=list(range(N_CORES)))
    out = np.stack([res.results[b]["out"] for b in range(B)])[:, None]
    return out.astype(np.float32)


# revision 9
# speedup vs baseline: 1.4104x; 1.0247x over previous
"""Affinity-propagate (SPN) Trainium2 Bass kernel, fp16 pipeline.

Computation (per batch element, see reference):
    w = g / conv3x3_ones(|g|)          # gates, [8, H, W], computed once
    d_{k+1} = max_c conv3x3_ones(w_c * d_k)   # 8 iterations

Distribution: pure data parallel, batch element b -> NeuronCore b (8 cores).

Per-core mapping (H=352 rows as 3 overlapping 128-row tiles):
  - All gate/depth data is fp16: DVE tensor_tensor ops run in 2x_1p mode
    (2 elem/cycle), halving the vector-engine time vs fp32.
  - p = w * d is ONE DVE mult per tile ([128, 8, WB], d broadcast over the
    channel axis; innermost stride 1 keeps the 2x mode).
  - 3x3 conv = tri-band matmul over the H/partition axis (fp16 stationary)
    x 3 PSUM-accumulated W-shifts; W is chunked 3x406 so each channel's
    conv occupies 3 PSUM banks and is evacuated PSUM->SBUF fp16 by ONE
    ScalarE copy (multi-bank AP), amortizing the per-op overhead.
  - channel max: 7 DVE tensor_max ops (fp16 2x), last one writes d in place.
  - seam rows between H tiles are fixed with 1-row SBUF->SBUF DMAs.
  - input g loads are spread across the Sync/Scalar/Vector DMA queues and
    staged 6 deep so the load pipeline stays ahead of gate normalization.
"""
from contextlib import ExitStack

import numpy as np

import concourse.bacc as bacc
import concourse.mybir as mybir
import concourse.tile as tile
from concourse.bass_utils import run_bass_kernel_spmd

F32 = mybir.dt.float32
F16 = mybir.dt.float16

B, C, H, W = 8, 8, 352, 1216
NCHUNK = 3
CW = 406                        # chunk width; 3 chunks of 406 = 1218 >= W
WB = NCHUNK * CW + 2            # 1220: [0]=pad, 1..1216 data, 1217+ pad
N_ITERS = 8
N_CORES = 8

ROW_BASE = [0, 126, 252]       # first global row of each H tile
ROWS = [128, 128, 100]         # partitions used by each H tile


def _build_nc():
    nc = bacc.Bacc("TRN2", target_bir_lowering=False, debug=False,
                   num_devices=N_CORES)
    g = nc.dram_tensor("g", [C, H, W], F32, kind="ExternalInput").ap()
    d_in = nc.dram_tensor("d", [H, W], F32, kind="ExternalInput").ap()
    band = nc.dram_tensor("band", [128, 128], F16, kind="ExternalInput").ap()
    out = nc.dram_tensor("out", [H, W], F32, kind="ExternalOutput").ap()

    with tile.TileContext(nc) as tc, ExitStack() as ctx:
        pw = ctx.enter_context(tc.tile_pool(name="w", bufs=1))
        pd = ctx.enter_context(tc.tile_pool(name="d", bufs=1))
        pc = ctx.enter_context(tc.tile_pool(name="const", bufs=1))
        pg = ctx.enter_context(tc.tile_pool(name="g32", bufs=6))
        pa = ctx.enter_context(tc.tile_pool(name="abs16", bufs=2))
        pr = ctx.enter_context(tc.tile_pool(name="r32", bufs=2))
        pw32 = ctx.enter_context(tc.tile_pool(name="w32", bufs=2))
        pp = ctx.enter_context(tc.tile_pool(name="p", bufs=2))
        pprop = ctx.enter_context(tc.tile_pool(name="prop", bufs=9))
        prm = ctx.enter_context(tc.tile_pool(name="rm", bufs=2))
        psum = ctx.enter_context(tc.tile_pool(name="psum", bufs=2,
                                              space="PSUM"))

        A = pc.tile([128, 128], F16, tag="band", name="bandt")
        nc.sync.dma_start(A[:], band[:])

        wt = [pw.tile([128, C, WB], F16, tag=f"w{t}", name=f"w{t}")
              for t in range(3)]
        dt_ = [pd.tile([128, WB], F16, tag=f"d{t}", name=f"d{t}")
               for t in range(3)]

        # ---- load depth (fp32 staging -> fp16), zero pads ----
        for t in range(3):
            R, rb = ROWS[t], ROW_BASE[t]
            nc.vector.memset(wt[t][:, :, 0:1], 0.0)
            nc.vector.memset(wt[t][:, :, W + 1:WB], 0.0)
            nc.vector.memset(dt_[t][:, 0:1], 0.0)
            nc.vector.memset(dt_[t][:, W + 1:WB], 0.0)
            d32 = pg.tile([128, W], F32, tag="g32", name="d32")
            nc.sync.dma_start(d32[0:R, :], d_in[rb:rb + R, :])
            nc.vector.tensor_copy(dt_[t][0:R, 1:W + 1], d32[0:R, :])

        # ---- phase 0: w = g / conv3x3_ones(|g|) ----
        dma_engines = [nc.sync, nc.scalar]
        for t in range(3):
            R, rb = ROWS[t], ROW_BASE[t]
            for c in range(C):
                g32 = pg.tile([128, W], F32, tag="g32", name="g32")
                dma_engines[(t * C + c) % 2].dma_start(
                    g32[0:R, :], g[c, rb:rb + R, :])
                a16 = pa.tile([128, WB], F16, tag="a16", name="a16")
                nc.vector.memset(a16[0:R, 0:1], 0.0)
                nc.vector.memset(a16[0:R, W + 1:WB], 0.0)
                nc.scalar.activation(a16[0:R, 1:W + 1], g32[0:R, :],
                                     mybir.ActivationFunctionType.Abs)
                ps = psum.tile([128, NCHUNK, 512], F32, tag="ps", name="ps")
                for k in range(NCHUNK):
                    for s in range(3):
                        nc.tensor.matmul(ps[0:R, k, 0:CW], A[0:R, 0:R],
                                         a16[0:R, k * CW + s:k * CW + s + CW],
                                         start=(s == 0), stop=(s == 2))
                r32 = pr.tile([128, NCHUNK, CW], F32, tag="r32", name="r32")
                nc.vector.reciprocal_approx_fast(
                    out=r32[0:R, :, :], in_=ps[0:R, :, 0:CW])
                rflat = r32[0:R, :, :].rearrange("p a b -> p (a b)")[:, 0:W]
                if c % 2 == 0:
                    # DVE path: fp32 mult straight into fp16 w
                    nc.vector.tensor_mul(wt[t][0:R, c, 1:W + 1],
                                         g32[0:R, :], rflat)
                else:
                    # GpSimd path: fp32 mult to staging, ScalarE converts;
                    # keeps the DVE free for the propagation iterations
                    w32 = pw32.tile([128, W], F32, tag="w32", name="w32")
                    nc.gpsimd.tensor_mul(w32[0:R, :], g32[0:R, :], rflat)
                    nc.scalar.copy(wt[t][0:R, c, 1:W + 1], w32[0:R, :])
        # w seam rows
        nc.sync.dma_start(wt[0][127:128, :, 1:W + 1], wt[1][1:2, :, 1:W + 1])
        nc.sync.dma_start(wt[1][0:1, :, 1:W + 1], wt[0][126:127, :, 1:W + 1])
        nc.sync.dma_start(wt[1][127:128, :, 1:W + 1], wt[2][1:2, :, 1:W + 1])
        nc.sync.dma_start(wt[2][0:1, :, 1:W + 1], wt[1][126:127, :, 1:W + 1])

        # ---- phase 1: 8 propagation iterations ----
        for k in range(N_ITERS):
            for t in range(3):
                R = ROWS[t]
                p16 = pp.tile([128, C, WB], F16, tag="p", name="p16")
                dbc = dt_[t][0:R, :].unsqueeze(1).broadcast_to([R, C, WB])
                nc.vector.tensor_mul(p16[0:R, :, :], wt[t][0:R, :, :], dbc)
                props = []
                for c in range(C):
                    ps = psum.tile([128, NCHUNK, 512], F32, tag="ps",
                                   name="ps")
                    for kk in range(NCHUNK):
                        for s in range(3):
                            nc.tensor.matmul(
                                ps[0:R, kk, 0:CW], A[0:R, 0:R],
                                p16[0:R, c, kk * CW + s:kk * CW + s + CW],
                                start=(s == 0), stop=(s == 2))
                    prop = pprop.tile([128, NCHUNK * CW], F16, tag="prop",
                                      name="prop")
                    nc.scalar.copy(
                        prop[0:R, :].rearrange("p (a b) -> p a b", a=NCHUNK),
                        ps[0:R, :, 0:CW])
                    props.append(prop)
                    # incremental channel max; last step writes d in place
                    if c == 1:
                        rm = prm.tile([128, W], F16, tag="rm", name="rm")
                        nc.vector.tensor_max(rm[0:R, :], props[0][0:R, 0:W],
                                             props[1][0:R, 0:W])
                    elif c in (2, 3, 4, 5, 6):
                        nc.vector.tensor_max(rm[0:R, :], rm[0:R, :],
                                             props[c][0:R, 0:W])
                    elif c == 7:
                        nc.vector.tensor_max(dt_[t][0:R, 1:W + 1],
                                             rm[0:R, :], props[7][0:R, 0:W])
            # seam rows
            nc.sync.dma_start(dt_[0][127:128, 1:W + 1], dt_[1][1:2, 1:W + 1])
            nc.sync.dma_start(dt_[1][0:1, 1:W + 1], dt_[0][126:127, 1:W + 1])
            nc.sync.dma_start(dt_[1][127:128, 1:W + 1], dt_[2][1:2, 1:W + 1])
            nc.sync.dma_start(dt_[2][0:1, 1:W + 1], dt_[1][126:127, 1:W + 1])

        # ---- output: fp16 -> fp32 staging -> HBM ----
        outspec = [(0, 0, 128), (1, 2, 128), (2, 2, 100)]
        for t, r0, r1 in outspec:
            o32 = pg.tile([128, W], F32, tag="g32", name="o32")
            nc.scalar.copy(o32[0:ROWS[t], :], dt_[t][0:ROWS[t], 1:W + 1])
            gb = ROW_BASE[t] + r0
            nc.sync.dma_start(out[gb:gb + (r1 - r0), :], o32[r0:r1, :])

    nc.compile()
    return nc


def _band_matrix():
    a = np.zeros((128, 128), dtype=np.float16)
    idx = np.arange(128)
    a[idx, idx] = 1.0
    a[idx[:-1], idx[:-1] + 1] = 1.0
    a[idx[1:], idx[1:] - 1] = 1.0
    return a


_NC_CACHE = None


def kernel(guidance: np.ndarray, blur_depth: np.ndarray) -> np.ndarray:
    """Full inputs in, full output out. Shards batch across 8 NeuronCores."""
    global _NC_CACHE
    guidance = np.asarray(guidance, dtype=np.float32)
    blur_depth = np.asarray(blur_depth, dtype=np.float32)
    assert guidance.shape == (B, C, H, W), guidance.shape
    assert blur_depth.shape == (B, 1, H, W), blur_depth.shape
    if _NC_CACHE is None:
        _NC_CACHE = _build_nc()
    nc = _NC_CACHE
    band = _band_matrix()
    in_maps = [
        {
            "g": np.ascontiguousarray(guidance[b], dtype=np.float32),
            "d": np.ascontiguousarray(blur_depth[b, 0], dtype=np.float32),
            "band": band,
        }
        for b in range(B)
    ]
    res = run_bass_kernel_spmd(nc, in_maps, core_ids=list(range(N_CORES)))
    out = np.stack([res.results[b]["out"] for b in range(B)])[:, None]
    return out.astype(np.float32)


# revision 11
# speedup vs baseline: 1.4238x; 1.0095x over previous
"""Affinity-propagate (SPN) Trainium2 Bass kernel, fp16 pipeline.

Computation (per batch element, see reference):
    w = g / conv3x3_ones(|g|)          # gates, [8, H, W], computed once
    d_{k+1} = max_c conv3x3_ones(w_c * d_k)   # 8 iterations

Distribution: pure data parallel, batch element b -> NeuronCore b (8 cores).

Per-core mapping (H=352 rows as 3 overlapping 128-row tiles):
  - All gate/depth data is fp16: DVE tensor_tensor ops run in 2x_1p mode
    (2 elem/cycle), halving the vector-engine time vs fp32.
  - p = w * d is ONE DVE mult per tile ([128, 8, WB], d broadcast over the
    channel axis; innermost stride 1 keeps the 2x mode).
  - 3x3 conv = tri-band matmul over the H/partition axis (fp16 stationary)
    x 3 PSUM-accumulated W-shifts; W is chunked 3x406 so each channel's
    conv occupies 3 PSUM banks and is evacuated PSUM->SBUF fp16 by ONE
    ScalarE copy (multi-bank AP), amortizing the per-op overhead.
  - channel max: 7 DVE tensor_max ops (fp16 2x), last one writes d in place.
  - seam rows between H tiles are fixed with 1-row SBUF->SBUF DMAs.
  - input g loads are spread across the Sync/Scalar/Vector DMA queues and
    staged 6 deep so the load pipeline stays ahead of gate normalization.
"""
from contextlib import ExitStack

import numpy as np

import concourse.bacc as bacc
import concourse.mybir as mybir
import concourse.tile as tile
from concourse.bass_utils import run_bass_kernel_spmd

F32 = mybir.dt.float32
F16 = mybir.dt.float16

B, C, H, W = 8, 8, 352, 1216
NCHUNK = 3
CW = 406                        # chunk width; 3 chunks of 406 = 1218 >= W
WB = NCHUNK * CW + 2            # 1220: [0]=pad, 1..1216 data, 1217+ pad
N_ITERS = 8
N_CORES = 8

ROW_BASE = [0, 126, 252]       # first global row of each H tile
ROWS = [128, 128, 100]         # partitions used by each H tile


def _build_nc():
    nc = bacc.Bacc("TRN2", target_bir_lowering=False, debug=False,
                   num_devices=N_CORES)
    g = nc.dram_tensor("g", [C, H, W], F32, kind="ExternalInput").ap()
    d_in = nc.dram_tensor("d", [H, W], F32, kind="ExternalInput").ap()
    band = nc.dram_tensor("band", [128, 128], F16, kind="ExternalInput").ap()
    out = nc.dram_tensor("out", [H, W], F32, kind="ExternalOutput").ap()

    with tile.TileContext(nc) as tc, ExitStack() as ctx:
        pw = ctx.enter_context(tc.tile_pool(name="w", bufs=1))
        pd = ctx.enter_context(tc.tile_pool(name="d", bufs=1))
        pc = ctx.enter_context(tc.tile_pool(name="const", bufs=1))
        pg = ctx.enter_context(tc.tile_pool(name="g32", bufs=6))
        pa = ctx.enter_context(tc.tile_pool(name="abs16", bufs=2))
        pr = ctx.enter_context(tc.tile_pool(name="r32", bufs=2))
        pw32 = ctx.enter_context(tc.tile_pool(name="w32", bufs=2))
        pp = ctx.enter_context(tc.tile_pool(name="p", bufs=2))
        pprop = ctx.enter_context(tc.tile_pool(name="prop", bufs=9))
        prm = ctx.enter_context(tc.tile_pool(name="rm", bufs=2))
        psum = ctx.enter_context(tc.tile_pool(name="psum", bufs=2,
                                              space="PSUM"))

        A = pc.tile([128, 128], F16, tag="band", name="bandt")
        nc.sync.dma_start(A[:], band[:])

        wt = [pw.tile([128, C, WB], F16, tag=f"w{t}", name=f"w{t}")
              for t in range(3)]
        dt_ = [pd.tile([128, WB], F16, tag=f"d{t}", name=f"d{t}")
               for t in range(3)]

        # ---- load depth (fp32 staging -> fp16), zero pads ----
        for t in range(3):
            R, rb = ROWS[t], ROW_BASE[t]
            nc.vector.memset(wt[t][:, :, 0:1], 0.0)
            nc.vector.memset(wt[t][:, :, W + 1:WB], 0.0)
            nc.vector.memset(dt_[t][:, 0:1], 0.0)
            nc.vector.memset(dt_[t][:, W + 1:WB], 0.0)
            d32 = pg.tile([128, W], F32, tag="g32", name="d32")
            nc.sync.dma_start(d32[0:R, :], d_in[rb:rb + R, :])
            nc.vector.tensor_copy(dt_[t][0:R, 1:W + 1], d32[0:R, :])

        # ---- phase 0: w = g / conv3x3_ones(|g|) ----
        # channel-PAIR-major order: each pair finishes (incl. its w seam
        # rows) early, unlocking iteration-1 matmuls for that pair while
        # later pairs still normalize.
        dma_engines = [nc.sync, nc.scalar]
        for pair in range(C // 2):
            for t in range(3):
                R, rb = ROWS[t], ROW_BASE[t]
                for c in (2 * pair, 2 * pair + 1):
                    g32 = pg.tile([128, W], F32, tag="g32", name="g32")
                    dma_engines[(t * C + c) % 2].dma_start(
                        g32[0:R, :], g[c, rb:rb + R, :])
                    a16 = pa.tile([128, WB], F16, tag="a16", name="a16")
                    nc.vector.memset(a16[0:R, 0:1], 0.0)
                    nc.vector.memset(a16[0:R, W + 1:WB], 0.0)
                    nc.scalar.activation(a16[0:R, 1:W + 1], g32[0:R, :],
                                         mybir.ActivationFunctionType.Abs)
                    ps = psum.tile([128, NCHUNK, 512], F32, tag="ps",
                                   name="ps")
                    for k in range(NCHUNK):
                        for s in range(3):
                            nc.tensor.matmul(
                                ps[0:R, k, 0:CW], A[0:R, 0:R],
                                a16[0:R, k * CW + s:k * CW + s + CW],
                                start=(s == 0), stop=(s == 2))
                    r32 = pr.tile([128, NCHUNK, CW], F32, tag="r32",
                                  name="r32")
                    nc.vector.reciprocal_approx_fast(
                        out=r32[0:R, :, :], in_=ps[0:R, :, 0:CW])
                    rflat = r32[0:R, :, :].rearrange(
                        "p a b -> p (a b)")[:, 0:W]
                    if c % 2 == 0:
                        # DVE path: fp32 mult straight into fp16 w
                        nc.vector.tensor_mul(wt[t][0:R, c, 1:W + 1],
                                             g32[0:R, :], rflat)
                    else:
                        # GpSimd path: fp32 mult to staging, ScalarE
                        # converts; keeps DVE free for the iterations
                        w32 = pw32.tile([128, W], F32, tag="w32", name="w32")
                        nc.gpsimd.tensor_mul(w32[0:R, :], g32[0:R, :], rflat)
                        nc.scalar.copy(wt[t][0:R, c, 1:W + 1], w32[0:R, :])
            # w seam rows for this channel pair
            c0, c1 = 2 * pair, 2 * pair + 2
            nc.sync.dma_start(wt[0][127:128, c0:c1, 1:W + 1],
                              wt[1][1:2, c0:c1, 1:W + 1])
            nc.sync.dma_start(wt[1][0:1, c0:c1, 1:W + 1],
                              wt[0][126:127, c0:c1, 1:W + 1])
            nc.sync.dma_start(wt[1][127:128, c0:c1, 1:W + 1],
                              wt[2][1:2, c0:c1, 1:W + 1])
            nc.sync.dma_start(wt[2][0:1, c0:c1, 1:W + 1],
                              wt[1][126:127, c0:c1, 1:W + 1])

        # ---- phase 1: 8 propagation iterations ----
        for k in range(N_ITERS):
            for t in range(3):
                R = ROWS[t]
                p16 = pp.tile([128, C, WB], F16, tag="p", name="p16")
                dbc2 = dt_[t][0:R, :].unsqueeze(1).broadcast_to([R, 2, WB])
                for pair in range(C // 2):
                    c0 = 2 * pair
                    nc.vector.tensor_mul(p16[0:R, c0:c0 + 2, :],
                                         wt[t][0:R, c0:c0 + 2, :], dbc2)
                props = []
                for c in range(C):
                    ps = psum.tile([128, NCHUNK, 512], F32, tag="ps",
                                   name="ps")
                    for kk in range(NCHUNK):
                        for s in range(3):
                            nc.tensor.matmul(
                                ps[0:R, kk, 0:CW], A[0:R, 0:R],
                                p16[0:R, c, kk * CW + s:kk * CW + s + CW],
                                start=(s == 0), stop=(s == 2))
                    prop = pprop.tile([128, NCHUNK * CW], F16, tag="prop",
                                      name="prop")
                    nc.scalar.copy(
                        prop[0:R, :].rearrange("p (a b) -> p a b", a=NCHUNK),
                        ps[0:R, :, 0:CW])
                    props.append(prop)
                    # incremental channel max; last step writes d in place
                    if c == 1:
                        rm = prm.tile([128, W], F16, tag="rm", name="rm")
                        nc.vector.tensor_max(rm[0:R, :], props[0][0:R, 0:W],
                                             props[1][0:R, 0:W])
                    elif c in (2, 3, 4, 5, 6):
                        nc.vector.tensor_max(rm[0:R, :], rm[0:R, :],
                                             props[c][0:R, 0:W])
                    elif c == 7:
                        nc.vector.tensor_max(dt_[t][0:R, 1:W + 1],
                                             rm[0:R, :], props[7][0:R, 0:W])
            # seam rows
            nc.sync.dma_start(dt_[0][127:128, 1:W + 1], dt_[1][1:2, 1:W + 1])
            nc.sync.dma_start(dt_[1][0:1, 1:W + 1], dt_[0][126:127, 1:W + 1])
            nc.sync.dma_start(dt_[1][127:128, 1:W + 1], dt_[2][1:2, 1:W + 1])
            nc.sync.dma_start(dt_[2][0:1, 1:W + 1], dt_[1][126:127, 1:W + 1])

        # ---- output: fp16 -> fp32 staging -> HBM ----
        outspec = [(0, 0, 128), (1, 2, 128), (2, 2, 100)]
        for t, r0, r1 in outspec:
            o32 = pg.tile([128, W], F32, tag="g32", name="o32")
            nc.scalar.copy(o32[0:ROWS[t], :], dt_[t][0:ROWS[t], 1:W + 1])
            gb = ROW_BASE[t] + r0
            nc.sync.dma_start(out[gb:gb + (r1 - r0), :], o32[r0:r1, :])

    nc.compile()
    return nc


def _band_matrix():
    a = np.zeros((128, 128), dtype=np.float16)
    idx = np.arange(128)
    a[idx, idx] = 1.0
    a[idx[:-1], idx[:-1] + 1] = 1.0
    a[idx[1:], idx[1:] - 1] = 1.0
    return a


_NC_CACHE = None


def kernel(guidance: np.ndarray, blur_depth: np.ndarray) -> np.ndarray:
    """Full inputs in, full output out. Shards batch across 8 NeuronCores."""
    global _NC_CACHE
    guidance = np.asarray(guidance, dtype=np.float32)
    blur_depth = np.asarray(blur_depth, dtype=np.float32)
    assert guidance.shape == (B, C, H, W), guidance.shape
    assert blur_depth.shape == (B, 1, H, W), blur_depth.shape
    if _NC_CACHE is None:
        _NC_CACHE = _build_nc()
    nc = _NC_CACHE
    band = _band_matrix()
    in_maps = [
        {
            "g": np.ascontiguousarray(guidance[b], dtype=np.float32),
            "d": np.ascontiguousarray(blur_depth[b, 0], dtype=np.float32),
            "band": band,
        }
        for b in range(B)
    ]
    res = run_bass_kernel_spmd(nc, in_maps, core_ids=list(range(N_CORES)))
    out = np.stack([res.results[b]["out"] for b in range(B)])[:, None]
    return out.astype(np.float32)
